# revision 45
# baseline (speedup 1.0000x reference)
"""LoOP (Local Outlier Probability) kernel for 8 TRN2 NeuronCores.

kernel(X, train_points) computes the reference nn_LoOP forward pass:
brute-force 20-NN of X over train_points, the 20-NN of each neighbor,
pdist ratios, and max(erf(lof/sqrt(2)), 0) -- distributed over 8 cores
(row-sharded train_points), with all compute on-device.
"""

import sys
import types
from contextlib import ExitStack

import numpy as np

import bass_rust
import concourse.bass as bass
import concourse.mybir as mybir
import concourse.tile as tile
from concourse.masks import make_identity
from concourse.tile import TileContext
from concourse.vector_clock import ScopedClock


# ---------------------------------------------------------------------------
# Toolchain workarounds: this walrus build accepts at most ONE sync wait per
# instruction (two for EventSemaphore), and the Tile kernel-tail drain
# collects one wait per outstanding sem domain. Split both.
# ---------------------------------------------------------------------------
def _split_multi_waits(nc):
    """This walrus build accepts at most ONE sync wait per instruction
    (two for EventSemaphore). Tile attaches as many waits as deps require.
    Rewrite: keep the first wait on the instruction, hoist extras onto
    same-engine NOPs inserted immediately before it."""
    edits = []
    for f in nc.m.functions:
        for bb in f.blocks:
            edits.append((bb, list(bb.instructions)))
    new_lists = []
    for bb, insts in edits:
        new = []
        changed = False
        for inst in insts:
            si = inst.sync_info
            cap = 2 if isinstance(inst, bass_rust.InstEventSemaphore) else 1
            if si is not None and si.on_wait and len(si.on_wait) > cap:
                waits = list(si.on_wait)
                for w in waits[cap:]:
                    nop = nc.engines[inst.engine].nop(nofuse=True).ins
                    nop.sync_info = bass_rust.SyncInfo(on_wait=[w],
                                                       on_update=[])
                    new.append(nop)
                inst.sync_info = bass_rust.SyncInfo(
                    on_wait=waits[:cap], on_update=list(si.on_update or []))
                changed = True
            new.append(inst)
        new_lists.append((bb, new, changed))
    for bb, new, changed in new_lists:
        if changed:
            bb.instructions = new


def _patched_drain_and_barrier(self, tick_clock, wait_clock):
    nc = self.nc
    _split_multi_waits(nc)
    drain_inst = nc.sync.drain()
    wait_clock.add_sem_waits(
        drain_inst.ins, ScopedClock({None: tick_clock.global_clock})
    )
    si = drain_inst.ins.sync_info
    if si is not None and si.on_wait and len(si.on_wait) > 1:
        waits = list(si.on_wait)
        upd = list(si.on_update or [])
        drain_inst.ins.sync_info = bass_rust.SyncInfo(
            on_wait=[waits[0]], on_update=upd
        )
        for w in waits[1:]:
            extra = nc.sync.drain()
            extra.ins.sync_info = bass_rust.SyncInfo(on_wait=[w], on_update=[])

    nc.all_engine_barrier()
    assert self.sems is not None
    popped = nc._tile_sem_poison_stack.pop()
    assert popped is self._sem_poison
    nc.clear_and_free_semaphores(list(self.sems.allocated().values()))
    nc.all_engine_barrier()


def install():
    TileContext._drain_and_barrier = _patched_drain_and_barrier
    try:
        _install_ntff_hook()
    except Exception:
        pass  # profiling hook is optional


def _install_ntff_hook():
    if "antenv.axon_hooks" in sys.modules:
        return
    mod = types.ModuleType("antenv.axon_hooks")
    state = {"hook": None}
    mod.set_axon_ntff_profile_hook = lambda h: state.__setitem__("hook", h)
    mod.get_axon_ntff_profile_hook = lambda: state["hook"]
    sys.modules["antenv.axon_hooks"] = mod
    import antenv

    antenv.axon_hooks = mod
    from trn_agent_boot.trn_boot import _ntff_profile_via_ctypes

    hook = _ntff_profile_via_ctypes("/opt/axon/libaxon_pjrt.so")
    if hook is not None:
        mod.set_axon_ntff_profile_hook(hook)




install()


F32 = mybir.dt.float32
BF16 = mybir.dt.bfloat16
U32 = mybir.dt.uint32
U16 = mybir.dt.uint16
AF = mybir.ActivationFunctionType
ALU = mybir.AluOpType

NC_N = 8          # cores
D = 512           # feature dim
K = 20            # neighbors
NT = 98           # tiles per core
NLOC = NT * 128   # 12544 rows per core (padded)
NPAD = NC_N * NLOC
PADV = 1.0e4      # padding row fill value
NEG = -3.0e38

SQ2I = 0.7071067811865476
TPI = 1.1283791670955126  # 2/sqrt(pi)


def _rounds_topk(nc, work, vals24, pos24, n_rounds=3):
    """max/match_replace rounds on `work` [P, F]; writes descending values
    into vals24 [P, 8*n_rounds] and positions into pos24 (uint32)."""
    for r in range(3):
        v8 = vals24[:, 8 * r:8 * r + 8]
        nc.vector.max(out=v8, in_=work)
        nc.vector.max_index(out=pos24[:, 8 * r:8 * r + 8], in_max=v8,
                            in_values=work)
        if r < n_rounds - 1:
            nc.vector.match_replace(out=work, in_to_replace=v8,
                                    in_values=work, imm_value=NEG)


def _rounds_topk_v(nc, work, vals, n_rounds=3):
    """Value-only top-(8*n_rounds): max/match_replace rounds, no index ops.
    Used with mantissa-packed scores where the index rides in the value."""
    for r in range(n_rounds):
        v8 = vals[:, 8 * r:8 * r + 8]
        nc.vector.max(out=v8, in_=work)
        if r < n_rounds - 1:
            nc.vector.match_replace(out=work, in_to_replace=v8,
                                    in_values=work, imm_value=NEG)


def _unpack_idx(nc, pool, packed, P, F, tagp="unp"):
    """packed [P, F] f32 whose low 16 bits hold the index; read them back
    via a strided u16 halfword view (exact -- values stay < 2^16)."""
    lo = packed[:].bitcast(U16).rearrange("p (f two) -> p f two", two=2)
    idx = pool.tile([P, F], U32, tag=tagp + "_i")
    nc.vector.tensor_copy(idx, lo[:, :, 0])
    return idx


def build(debug=False, stage=99):
    nc = bass.Bass()
    tp = nc.declare_dram_parameter("tp", [NLOC, D], BF16, isOutput=False)
    x_in = nc.declare_dram_parameter("x", [1, D], F32, isOutput=False)
    tpf = nc.declare_dram_parameter("tpf", [NPAD, D], F32, isOutput=False)
    base_in = nc.declare_dram_parameter("base", [1, 1], F32, isOutput=False)
    out = nc.declare_dram_parameter("out", [1, 1], F32, isOutput=True)
    if debug:
        dbg_d0 = nc.declare_dram_parameter("dbg_d0", [128, NT], F32, isOutput=True)
        dbg_nl24 = nc.declare_dram_parameter("dbg_nl24", [24, 1], U32, isOutput=True)
        dbg_nb = nc.declare_dram_parameter("dbg_nb", [K, D], F32, isOutput=True)
        dbg_s2 = nc.declare_dram_parameter("dbg_s2", [K, 1], F32, isOutput=True)

    with tile.TileContext(nc) as tc, ExitStack() as ctx:
        # ---- pools ----
        consts = ctx.enter_context(tc.tile_pool(name="consts", bufs=1))
        big = ctx.enter_context(tc.tile_pool(name="big", bufs=1))
        ld = ctx.enter_context(tc.tile_pool(name="ld", bufs=6))
        bfp = ctx.enter_context(tc.tile_pool(name="bfp", bufs=4))
        scrp = ctx.enter_context(tc.tile_pool(name="scrp", bufs=2))
        small = ctx.enter_context(tc.tile_pool(name="small", bufs=1))
        gat = ctx.enter_context(tc.tile_pool(name="gat", bufs=2))
        psum_t = ctx.enter_context(tc.tile_pool(name="psum_t", bufs=2, space="PSUM"))
        psum_a = ctx.enter_context(tc.tile_pool(name="psum_a", bufs=2, space="PSUM"))
        psum_c = ctx.enter_context(tc.tile_pool(name="psum_c", bufs=2, space="PSUM"))
        psum_s = ctx.enter_context(tc.tile_pool(name="psum_s", bufs=1, space="PSUM"))
        dram = ctx.enter_context(tc.tile_pool(name="dram", bufs=1, space="DRAM"))

        # ---- constants ----
        ident = consts.tile([128, 128], BF16)
        make_identity(nc, ident)
        iota_pu = consts.tile([128, 1], U32)
        nc.gpsimd.iota(iota_pu, pattern=[[0, 1]], base=0, channel_multiplier=1)
        iota_n = consts.tile([128, NT], U16)
        nc.gpsimd.iota(iota_n, pattern=[[128, NT]], base=0,
                       channel_multiplier=1)
        iota192 = consts.tile([1, NC_N * 24], U16)
        nc.gpsimd.iota(iota192, pattern=[[1, NC_N * 24]], base=0,
                       channel_multiplier=0)
        iota_p = consts.tile([128, 1], F32)
        nc.vector.tensor_copy(iota_p, iota_pu)
        ones20 = consts.tile([K, 1], F32)
        nc.vector.memset(ones20, 1.0)
        bglob = consts.tile([128, 1], F32)
        nc.sync.dma_start(bglob, base_in[0:1, :].to_broadcast([128, 1]))

        # X broadcast: [1, D] -> [128, D] fp32 (DMA with repeated reads)
        xbf = consts.tile([128, D], F32)
        nc.sync.dma_start(xbf, x_in[0:1, :].to_broadcast([128, D]))
        # bf16 X copies for the bf16 streaming path
        xbb = consts.tile([128, D], BF16)
        nc.vector.tensor_copy(xbb, xbf)
        xbb2 = consts.tile([128, 2 * D], BF16)
        nc.vector.tensor_copy(xbb2[:, 0:D], xbf)
        nc.vector.tensor_copy(xbb2[:, D:2 * D], xbf)

        # ---- persistent buffers ----
        stash = big.tile([128, NT, 4, 128], BF16)   # (t - X)^T bf16
        d0buf = big.tile([128, NT], F32)            # ||t - X||^2
        sbufC = big.tile([128, NT, K], F32)

        # ================= PHASE A =================
        # paired tiles: one DMA + one subtract per 256 rows
        NP2 = NT // 2
        for j in range(NP2):
            tl = ld.tile([128, 2, D], BF16, tag="tl")
            nc.sync.dma_start(
                tl, tp[j * 256:(j + 1) * 256, :].rearrange(
                    "(t p) d -> p t d", t=2, p=128))
            # u = t - X in bf16 (DVE as two 512-wide ops; GPS whole pair)
            ub = bfp.tile([128, 2, D], BF16, tag="ub")
            if j % 3 == 0:
                nc.vector.tensor_tensor(out=ub[:, 0], in0=tl[:, 0],
                                        in1=xbb, op=ALU.subtract)
                nc.vector.tensor_tensor(out=ub[:, 1], in0=tl[:, 1],
                                        in1=xbb, op=ALU.subtract)
            else:
                nc.gpsimd.tensor_tensor(
                    out=ub[:].rearrange("p t d -> p (t d)"),
                    in0=tl[:].rearrange("p t d -> p (t d)"),
                    in1=xbb2, op=ALU.subtract)
            # d0^2: ACT square+accum (scratch out to PSUM, off SBUF)
            scr0 = psum_a.tile([128, D], F32, tag="scr")
            nc.scalar.activation(scr0, ub[:, 0], AF.Square,
                                 accum_out=d0buf[:, 2 * j:2 * j + 1])
            scr1 = psum_a.tile([128, D], F32, tag="scr")
            nc.scalar.activation(scr1, ub[:, 1], AF.Square,
                                 accum_out=d0buf[:, 2 * j + 1:2 * j + 2])
            # transpose to stash
            ps = psum_t.tile([128, 2, 4, 128], BF16, tag="ps")
            for t in range(2):
                for c in range(4):
                    nc.tensor.transpose(ps[:, t, c, :],
                                        ub[:, t, c * 128:(c + 1) * 128],
                                        ident)
            nc.vector.tensor_copy(stash[:, 2 * j:2 * j + 2], ps)

        # selection score = -d0^2 (f32; low 16 bits overwritten by index)
        selq = small.tile([128, NT], F32)
        nc.vector.tensor_scalar_mul(selq, d0buf, -1.0)
        if debug:
            nc.sync.dma_start(dbg_d0[:, :], d0buf)

        if stage < 2:
            nc.sync.dma_start(out[:, :], d0buf[0:1, 0:1])
            return nc
        # ================= PHASE B =================
        # pack: overwrite each score's low 16 bits with the local row idx
        nc.vector.tensor_copy(
            selq[:].bitcast(U16).rearrange("p (f two) -> p f two",
                                           two=2)[:, :, 0],
            iota_n)
        qv8 = small.tile([128, 8], F32)
        nc.vector.max(out=qv8, in_=selq)
        qv1k = small.tile([1, 1024], F32)
        nc.sync.dma_start(qv1k, qv8)
        qv24 = small.tile([1, 24], F32)
        _rounds_topk_v(nc, qv1k, qv24)
        # -> partitions; unpack local idx; make global; gather rows
        q24P = small.tile([24, 1], F32)
        nc.sync.dma_start(q24P, qv24)
        nli = _unpack_idx(nc, small, q24P, 24, 1, tagp="qunp")
        nlf = small.tile([24, 1], F32)
        nc.vector.tensor_copy(nlf, nli)
        nc.vector.tensor_scalar(out=nlf, in0=nlf, scalar1=bglob[0:24, 0:1],
                                scalar2=None, op0=ALU.add)
        nl24 = small.tile([24, 1], U32)
        nc.vector.tensor_copy(nl24, nlf)
        if debug:
            nc.sync.dma_start(dbg_nl24[:, :], nl24)
        if stage < 3:
            nc.sync.dma_start(out[:, :], nlf[0:1, 0:1])
            return nc
        # allgather candidates: [24, 2] = packed approx -d0^2 | global idx
        cc_in = dram.tile([24, 2], F32)
        nc.sync.dma_start(cc_in[:, 0:1], q24P)
        nc.sync.dma_start(cc_in[:, 1:2], nlf)
        gath = dram.tile([NC_N * 24, 2], F32, addr_space="Shared")
        nc.gpsimd.collective_compute(
            "AllGather", ALU.bypass,
            replica_groups=[list(range(NC_N))],
            ins=[cc_in.opt()], outs=[gath.opt()])
        # merge: top-20 by approx packed score; repack low bits = position
        gv = small.tile([1, NC_N * 24], F32)
        nc.sync.dma_start(gv, gath[:, 0:1])
        nc.vector.tensor_copy(
            gv[:].bitcast(U16).rearrange("p (f two) -> p f two",
                                         two=2)[:, :, 0],
            iota192)
        gv24 = small.tile([1, 24], F32)
        _rounds_topk_v(nc, gv, gv24)
        # positions of the top-20 -> partitions; fetch global idx column
        g24P = small.tile([24, 1], F32)
        nc.sync.dma_start(g24P, gv24)
        gposi = _unpack_idx(nc, small, g24P, 24, 1, tagp="gunp")
        gi2 = small.tile([K, 2], F32)
        nc.gpsimd.indirect_dma_start(
            out=gi2, out_offset=None, in_=gath[:, :],
            in_offset=bass.IndirectOffsetOnAxis(ap=gposi[0:K, 0:1], axis=0))
        gidx = small.tile([K, 1], U32)
        nc.vector.tensor_copy(gidx, gi2[:, 1:2])
        nbrow = small.tile([K, D], F32)
        nc.gpsimd.indirect_dma_start(
            out=nbrow, out_offset=None, in_=tpf[:, :],
            in_offset=bass.IndirectOffsetOnAxis(ap=gidx[:, 0:1], axis=0))
        nb_dram = dram.tile([K, D], F32)
        nc.sync.dma_start(nb_dram, nbrow)
        if debug:
            nc.sync.dma_start(dbg_nb[:, :], nbrow)
        # nbT = 2*(nb - X) in bf16: [128, 4, K]
        unb = small.tile([K, D], F32)
        nc.vector.tensor_tensor(out=unb, in0=nbrow, in1=xbf[0:K, :],
                                op=ALU.subtract)
        # exact d0^2 of the selected 20 -> sd0p = sum (for pdist_x)
        dscr20 = small.tile([K, D], F32)
        d0x20 = small.tile([K, 1], F32)
        nc.scalar.activation(dscr20, unb, AF.Square, accum_out=d0x20)
        psd0 = psum_s.tile([1, 1], F32, tag="ps1")
        nc.tensor.matmul(psd0, lhsT=ones20, rhs=d0x20, start=True, stop=True)
        sd0p = small.tile([1, 1], F32)
        nc.vector.tensor_copy(sd0p, psd0)
        nbb = small.tile([K, D], BF16)
        nc.vector.tensor_scalar_mul(nbb, unb, 2.0)
        psn = psum_s.tile([128, 4, K], BF16)
        for c in range(4):
            nc.tensor.transpose(psn[:, c, :], nbb[:, c * 128:(c + 1) * 128],
                                ident[0:K, 0:K])
        nbT = small.tile([128, 4, K], BF16)
        nc.vector.tensor_copy(nbT, psn)

        if stage < 4:
            nc.gpsimd.dma_start(out[:, :], nbT[0:1, 0, 0:1])
            return nc
        # ================= PHASE C =================
        for j in range(NT // 2):
            f = 2 * j
            psc = psum_c.tile([128, 2, K], F32, tag="psc")
            for t in range(2):
                for c in range(4):
                    nc.tensor.matmul(psc[:, t, :],
                                     lhsT=stash[:, f + t, c, :],
                                     rhs=nbT[:, c, :],
                                     start=(c == 0), stop=(c == 3))
            # s = 2*dot - tn2 (both tiles in one drain op)
            nc.vector.tensor_tensor(
                out=sbufC[:, f:f + 2, :], in0=psc,
                in1=d0buf[:, f:f + 2].rearrange(
                    "p (t o) -> p t o", t=2, o=1).to_broadcast([128, 2, K]),
                op=ALU.subtract)

        # pack: overwrite each score's low 16 bits with the local row idx
        nc.vector.tensor_copy(
            sbufC[:].bitcast(U16).rearrange(
                "p f (k two) -> p f k two", two=2)[:, :, :, 0],
            iota_n[:].rearrange("p (f o) -> p f o", f=NT, o=1
                                ).to_broadcast([128, NT, K]))
        # per-row top-8 per partition (packed; no index ops)
        cv8 = small.tile([128, K, 8], F32)
        for k in range(K):
            nc.vector.max(out=cv8[:, k, :], in_=sbufC[:, :, k])
        # rearrange [128, K, 8] -> [K, 1024] via DRAM bounce
        cvd = dram.tile([128, K * 8], F32)
        nc.sync.dma_start(cvd, cv8)
        cvM = small.tile([K, 1024], F32)
        nc.sync.dma_start(
            cvM, cvd[:].rearrange("p (k j) -> k p j", k=K, j=8))
        if stage < 5:
            nc.sync.dma_start(out[:, :], cvM[0:1, 0:1])
            return nc
        # local merge to top-24 per row; unpack local idx -> global idx
        cv24 = small.tile([K, 24], F32)
        _rounds_topk_v(nc, cvM, cv24)
        cgi = _unpack_idx(nc, small, cv24, K, 24, tagp="cunp")
        cgf = small.tile([K, 24], F32)
        nc.vector.tensor_copy(cgf, cgi)
        nc.vector.tensor_scalar(out=cgf, in0=cgf, scalar1=bglob[0:K, 0:1],
                                scalar2=None, op0=ALU.add)
        cgu = small.tile([K, 24], U32)
        nc.vector.tensor_copy(cgu, cgf)
        # bounce to DRAM flat [480], reload as [120, 4] chunk-major
        cf_dram = dram.tile([K * 24, 1], U32)
        nc.sync.dma_start(cf_dram, cgu)
        cfP = small.tile([120, 4], U32)
        nc.sync.dma_start(
            cfP, cf_dram[:].rearrange("(c p) o -> p (c o)", c=4, p=120))
        # gather candidate rows; nb rows are a fixed broadcast pattern
        # (partition p of chunk c holds candidate for nb row 5c + p//24)
        dn2 = small.tile([120, 4], F32)
        for c in range(4):
            rows = gat.tile([120, D], F32, tag="rows")
            nc.gpsimd.indirect_dma_start(
                out=rows, out_offset=None, in_=tpf[:, :],
                in_offset=bass.IndirectOffsetOnAxis(ap=cfP[:, c:c + 1], axis=0))
            nbr = gat.tile([120, D], F32, tag="nbr")
            for g in range(5):
                nc.sync.dma_start(
                    nbr[24 * g:24 * (g + 1), :],
                    nb_dram[5 * c + g:5 * c + g + 1, :].to_broadcast([24, D]))
            du = gat.tile([120, D], F32, tag="du")
            nc.vector.tensor_tensor(out=du, in0=rows, in1=nbr,
                                    op=ALU.subtract)
            dscr = gat.tile([120, D], F32, tag="dscr")
            nc.scalar.activation(dscr, du, AF.Square,
                                 accum_out=dn2[:, c:c + 1])
        if stage < 6:
            nc.sync.dma_start(out[:, :], dn2[0:1, 0:1])
            return nc
        # back to [K, 24] (negated for max)
        dn_dram = dram.tile([K * 24, 1], F32)
        nc.sync.dma_start(
            dn_dram[:].rearrange("(c p) o -> p (c o)", c=4, p=120), dn2)
        dnM = small.tile([K, 24], F32)
        nc.sync.dma_start(dnM, dn_dram)
        nc.vector.tensor_scalar_mul(dnM, dnM, -1.0)
        dv24 = small.tile([K, 24], F32)
        _rounds_topk_v(nc, dnM, dv24)
        # local top-20 exact values -> allgather [K, K]
        c2_in = dram.tile([K, K], F32)
        nc.sync.dma_start(c2_in, dv24[:, 0:K])
        gath2 = dram.tile([NC_N * K, K], F32, addr_space="Shared")
        nc.gpsimd.collective_compute(
            "AllGather", ALU.bypass,
            replica_groups=[list(range(NC_N))],
            ins=[c2_in.opt()], outs=[gath2.opt()])
        # merge per row: [K, 8*K]
        g2 = small.tile([K, NC_N * K], F32)
        nc.sync.dma_start(
            g2, gath2[:].rearrange("(j k) m -> k j m", j=NC_N, k=K))
        g2v = small.tile([K, 24], F32)
        _rounds_topk_v(nc, g2, g2v)
        # S2[k] = sum of top-20 exact dist^2 (values negated)
        s2 = small.tile([K, 1], F32)
        nc.vector.tensor_reduce(out=s2, in_=g2v[:, 0:K],
                                axis=mybir.AxisListType.X, op=ALU.add)
        nc.vector.tensor_scalar_mul(s2, s2, -1.0 / K)
        if debug:
            nc.sync.dma_start(dbg_s2[:, :], s2)

        if stage < 7:
            nc.sync.dma_start(out[:, :], s2[0:1, 0:1])
            return nc
        # ================= PHASE D =================
        # pdist_nb = sqrt(s2) ; nf = sum over the 20 rows (PE ones-matmul)
        pd = small.tile([K, 1], F32)
        nc.scalar.activation(pd, s2, AF.Sqrt)
        psd = psum_s.tile([1, 1], F32, tag="ps1")
        nc.tensor.matmul(psd, lhsT=ones20, rhs=pd, start=True, stop=True)
        nf = small.tile([1, 1], F32)
        nc.vector.tensor_copy(nf, psd)
        # pdist_x = sqrt(sd0p/20)
        px = small.tile([1, 1], F32)
        nc.scalar.activation(px, sd0p, AF.Sqrt, scale=1.0 / K)
        # lof = px/nf*K - 1 ; out = relu(erf(lof/sqrt(2)))
        rnf = small.tile([1, 1], F32)
        nc.vector.reciprocal(rnf, nf)
        z = small.tile([1, 1], F32)
        nc.vector.tensor_tensor(out=z, in0=px, in1=rnf, op=ALU.mult)
        nc.vector.tensor_scalar(out=z, in0=z, scalar1=float(K),
                                scalar2=-1.0, op0=ALU.mult, op1=ALU.add)
        ef = small.tile([1, 1], F32)
        nc.scalar.activation(ef, z, AF.Erf, scale=SQ2I)
        res = small.tile([1, 1], F32)
        nc.scalar.activation(res, ef, AF.Relu)
        nc.sync.dma_start(out[:, :], res)

    return nc


def prepare_inputs(X, train_points):
    """Pad + shard the full inputs into per-core in_maps."""
    X = np.ascontiguousarray(X, dtype=np.float32)
    tpts = np.ascontiguousarray(train_points, dtype=np.float32)
    n = tpts.shape[0]
    pad = np.full((NPAD - n, D), PADV, dtype=np.float32)
    tpad = np.concatenate([tpts, pad], axis=0)
    import ml_dtypes

    tpad_bf = tpad.astype(ml_dtypes.bfloat16)
    in_maps = []
    for i in range(NC_N):
        in_maps.append({
            "tp": np.ascontiguousarray(tpad_bf[i * NLOC:(i + 1) * NLOC]),
            "x": X.reshape(1, D),
            "tpf": tpad,
            "base": np.full((1, 1), float(i * NLOC), dtype=np.float32),
        })
    return in_maps


_NC_CACHE = {}


def kernel(X, train_points):
    from concourse.bass_utils import run_bass_kernel_spmd

    if "nc" not in _NC_CACHE:
        _NC_CACHE["nc"] = build(debug=False)
    nc = _NC_CACHE["nc"]
    in_maps = prepare_inputs(X, train_points)
    res = run_bass_kernel_spmd(nc, in_maps, list(range(NC_N)), trace=False)
    out = np.asarray(res.results[0]["out"], dtype=np.float32).reshape(())
    return out

